# revision 36
# baseline (speedup 1.0000x reference)
"""Bayesian (VDP) multi-head self-attention forward on 8 Trainium2 NeuronCores.

Sharding: core = (batch b, query-half). The host permutes each batch's
sequence so the core-local 512 query rows come first (attention sums over
keys, so a consistent key permutation is a no-op). Each core computes its
512 output rows completely: Q/K/V variational linears, 12-head VDP
attention, and the output projection including softplus. Host glue is
concatenation plus the scalar KL term.

Device layout notes:
- Inputs arrive pre-transposed [768, S] so every matmul contracts over the
  partition dim with zero on-device transposes.
- Attention scores are produced directly in [k, q] orientation; softmax
  normalization is algebraically deferred to per-q post-scales applied to
  the [d, q]-oriented PV outputs (the 1/r powers commute with the k-sum).
- Sigma_weights = (W - W^2)^2 * sigma_score/(d*wdim) is computed as
  (E*(r-E))^2 * ss/(d*wdim) * r^-4 with E = exp(score/sqrt(d)) unnormalized.
- mu-path matmuls run in float32r (full-rate fp32), sigma path in bf16.
"""

import contextlib
import numpy as np
import ml_dtypes

B, S, E, H, d = 4, 1024, 768, 12, 64
LOC = 512
NC = 8
INV_DWDIM = float(1.0 / (64.0 * 1024.0))
ET = E // 128   # 6 embedding partition tiles
ST = S // 128   # 8 sequence partition tiles
HT = 2          # heads per 128-partition tile

_CACHE: dict = {}


def _softplus64(x):
    x = np.asarray(x, np.float64)
    return np.log1p(np.exp(-np.abs(x))) + np.maximum(x, 0.0)


def _build_program():
    import concourse.bass as bass
    import concourse.bacc as bacc
    import concourse.mybir as mybir
    import concourse.tile as tile

    f32 = mybir.dt.float32
    f32r = mybir.dt.float32r
    bf16 = mybir.dt.bfloat16
    AF = mybir.ActivationFunctionType
    OP = mybir.AluOpType
    PSUM = bass.MemorySpace.PSUM

    nc = bacc.Bacc("TRN2", target_bir_lowering=False, debug=False,
                   num_devices=NC)

    xmT_d = nc.dram_tensor("xmT", [E, S], f32r, kind="ExternalInput")
    xsT_d = nc.dram_tensor("xsT", [E, S], bf16, kind="ExternalInput")
    w_d = {k: nc.dram_tensor(f"w{k}", [E, E], f32r, kind="ExternalInput") for k in "qkvo"}
    wsq_d = {k: nc.dram_tensor(f"w{k}s", [E, E], bf16, kind="ExternalInput") for k in "qkvo"}
    sig_d = {k: nc.dram_tensor(f"sig{k}", [1, E], bf16, kind="ExternalInput")
             for k in "qkvo"}
    muo_d = nc.dram_tensor("mu_out", [LOC, E], f32, kind="ExternalOutput")
    sgo_d = nc.dram_tensor("sig_out", [LOC, E], f32, kind="ExternalOutput")

    _actchain = {"last": None}

    def act_phase(insts):
        """Order a batch of same-table-set ACT ops after the previous batch
        (scheduler-only edges) so walrus emits one table load per batch."""
        if not insts:
            return
        if _actchain["last"] is not None:
            prev = _actchain["last"].ins
            for bi in insts:
                tile.add_dep_helper(bi.ins, prev, sync=False, reason="act-set phase")
        _actchain["last"] = insts[-1]

    def hs(tiles, h, cols=None):
        """AP for head h (64 rows) of a [768, *]-as-6-tiles tensor."""
        t, p0 = h // HT, (h % HT) * 64
        ap = tiles[t]
        return ap[p0:p0 + 64, :] if cols is None else ap[p0:p0 + 64, cols]

    with nc.allow_low_precision(reason="f32r/bf16 compute by design"), \
         tile.TileContext(nc) as tc, contextlib.ExitStack() as ex:
        cst = ex.enter_context(tc.tile_pool(name="cst", bufs=1))

        ones_f = cst.tile([1, 128], f32, tag="ones_f", name="ones_f")
        nc.vector.memset(ones_f[:], 1.0)
        ones_cb = cst.tile([128, 1], bf16, tag="ones_cb", name="ones_cb")
        nc.vector.memset(ones_cb[:], 1.0)
        o768_cb = cst.tile([128, 1], bf16, tag="o768_cb", name="o768_cb")
        nc.vector.memset(o768_cb[:], float(1.0 / 768.0))
        wsig = {}
        for k in "qkvo":
            wsig[k] = cst.tile([1, E], bf16, tag=f"wsig{k}", name=f"wsig{k}")
            nc.sync.dma_start(wsig[k][:], sig_d[k][:])
        combo_t = cst.tile([1, S], bf16, tag="combo_t", name="combo_t")
        combo = combo_t[0:1, :]
        combo_o_t = cst.tile([1, LOC], bf16, tag="combo_o_t", name="combo_o_t")
        combo_o = combo_o_t[0:1, :]

        perA = ex.enter_context(tc.tile_pool(name="perA", bufs=1))
        mu_qT = [perA.tile([128, LOC], f32r, tag=f"muq{i}", name=f"muq{i}") for i in range(ET)]
        mu_kT = [perA.tile([128, S], f32r, tag=f"muk{i}", name=f"muk{i}") for i in range(ET)]
        # per-head stacked sigma operands: rows 0:64 = sg_k(h), 64:128 = sg_q(h)
        skq = [perA.tile([128, S], bf16, tag=f"skq{h}", name=f"skq{h}") for h in range(H)]
        # per-head stacked rhs: rows 0:64 = mu_q^2/64 + sg_q/64, 64:128 = mu_k^2/64
        qb = [perA.tile([128, LOC], bf16, tag=f"qb{h}", name=f"qb{h}") for h in range(H)]



        muv1 = [perA.tile([128, H * 65], bf16, tag=f"muv1_{i}", name=f"muv1_{i}") for i in range(ST)]
        V1 = [perA.tile([128, E], bf16, tag=f"V1_{i}", name=f"V1_{i}") for i in range(ST)]
        V2 = [perA.tile([128, E], bf16, tag=f"V2_{i}", name=f"V2_{i}") for i in range(ST)]

        # ================= stage 1: linears =================
        with tc.tile_pool(name="s1x", bufs=1) as s1x, \
             tc.tile_pool(name="s1w", bufs=1) as s1w, \
             tc.tile_pool(name="s1wb", bufs=1) as s1wb, \
             tc.tile_pool(name="s1p", bufs=1, space=PSUM) as s1p, \
             tc.tile_pool(name="s1s", bufs=3) as s1s:

            xm_big = [s1x.tile([128, 2 * S], f32r, tag=f"xmB{j}", name=f"xmB{j}")
                      for j in range(3)]
            xs_big = [s1x.tile([128, 3 * S], bf16, tag=f"xsB{j}", name=f"xsB{j}")
                      for j in range(2)]
            xmT = [xm_big[i // 2][:, (i % 2) * S:(i % 2 + 1) * S] for i in range(ET)]
            xsT = [xs_big[i // 3][:, (i % 3) * S:(i % 3 + 1) * S] for i in range(ET)]
            xmT_r = xmT_d.rearrange("(t p) s -> p t s", p=128)
            xsT_r = xsT_d.rearrange("(t p) s -> p t s", p=128)
            for j in range(3):
                nc.sync.dma_start(
                    xm_big[j][:].rearrange("p (t s) -> p t s", s=S),
                    xmT_r[:, 2 * j:2 * j + 2, :])
            for j in range(2):
                nc.sync.dma_start(
                    xs_big[j][:].rearrange("p (t s) -> p t s", s=S),
                    xsT_r[:, 3 * j:3 * j + 3, :])

            # combo[s] = sum_f (xm[f,s]/768)^2 + sum_f xs[f,s]/768
            for sc in range(2):
                scs = slice(sc * 512, (sc + 1) * 512)
                csum = s1p.tile([1, 512], f32, tag="csum", name="csum", bufs=1)
                for i in range(ET):
                    x2 = s1s.tile([128, 512], bf16, tag="x2", name="x2")
                    nc.scalar.activation(x2[:], xmT[i][:, scs], AF.Square,
                                         scale=float(1.0 / 768.0))
                    nc.tensor.matmul(csum[:], ones_cb[:], x2[:],
                                     start=(i == 0), stop=False)
                for i in range(ET):
                    nc.tensor.matmul(csum[:], o768_cb[:], xsT[i][:, scs],
                                     start=False, stop=(i == ET - 1))
                nc.vector.tensor_copy(combo[0:1, scs], csum[:])

            wt_big = [s1w.tile([128, 3 * E], f32r, tag=f"wB{j}", name=f"wB{j}")
                      for j in range(2)]
            wtb_big = s1wb.tile([128, ET * E], bf16, tag="wsB", name="wsB")
            wt = [wt_big[i // 3][:, (i % 3) * E:(i % 3 + 1) * E] for i in range(ET)]
            wtb = [wtb_big[:, i * E:(i + 1) * E] for i in range(ET)]

            def load_weights(k):
                wr = w_d[k].rearrange("(t p) e -> p t e", p=128)
                wbr = wsq_d[k].rearrange("(t p) e -> p t e", p=128)
                for j in range(2):
                    nc.sync.dma_start(
                        wt_big[j][:].rearrange("p (t e) -> p t e", e=E),
                        wr[:, 3 * j:3 * j + 3, :])
                nc.sync.dma_start(
                    wtb_big[:].rearrange("p (t e) -> p t e", e=E), wbr[:])

            # ---- Q/K transposed-out linears ----
            s1qk_cm = tc.tile_pool(name="s1qk", bufs=1)
            s1qk = s1qk_cm.__enter__()
            for k, mu_dst, nq in (("q", mu_qT, LOC // 512), ("k", mu_kT, S // 512)):
                load_weights(k)
                for et in range(ET):
                    ecs = slice(et * 128, (et + 1) * 128)
                    for sc in range(nq):
                        scs = slice(sc * 512, (sc + 1) * 512)
                        ps = s1p.tile([128, 512], f32, tag="lin", name="lin", bufs=3)
                        for ft in range(ET):
                            nc.tensor.matmul(ps[:], wt[ft][:, ecs],
                                             xmT[ft][:, scs],
                                             start=(ft == 0), stop=(ft == ET - 1))
                        nc.vector.tensor_copy(mu_dst[et][:, scs], ps[:])
                dst_half = slice(64, 128) if k == "q" else slice(0, 64)
                spx_l = []
                _exps = []
                for et in range(ET):
                    ecs = slice(et * 128, (et + 1) * 128)
                    for sc in range(S // 512):
                        scs = slice(sc * 512, (sc + 1) * 512)
                        ps = s1p.tile([128, 512], f32, tag="lin", name="lin", bufs=3)
                        for ft in range(ET):
                            nc.tensor.matmul(ps[:], wtb[ft][:, ecs], xsT[ft][:, scs],
                                             start=(ft == 0), stop=False)
                        nc.tensor.matmul(ps[:], wsig[k][0:1, ecs],
                                         combo[0:1, scs],
                                         start=False, stop=True)
                        spx = s1qk.tile([128, 512], bf16, tag=f"spx{et}_{sc}",
                                        name=f"spx{et}_{sc}", bufs=1)
                        _exps.append(nc.scalar.activation(spx[:], ps[:], AF.Exp))
                        spx_l.append((spx, et, scs))
                act_phase(_exps)
                _lns = []
                for spx, et, scs in spx_l:
                    for par in range(2):
                        _lns.append(nc.scalar.activation(
                            skq[2 * et + par][dst_half, scs],
                            spx[par * 64:(par + 1) * 64, :], AF.Ln, bias=1.0))
                act_phase(_lns)

            s1qk_cm.__exit__(None, None, None)

            # ---- V natural-out linears ----
            load_weights("v")
            s1v_cm = tc.tile_pool(name="s1v", bufs=1)
            s1v = s1v_cm.__enter__()

            def flush_vdefer(vdefer):
                global_lns = []
                act_phase(_vexps)
                _vexps.clear()
                for spv, st, ecs, t2 in vdefer:
                    sgv = s1s.tile([128, 384], bf16, tag="sgv", name="sgv", bufs=3)
                    global_lns.append(
                        nc.scalar.activation(sgv[:], spv[:], AF.Ln, bias=1.0))
                    nc.vector.tensor_scalar_mul(V1[st][:, ecs], sgv[:],
                                                float(1.0 / 1024.0))
                    nc.vector.scalar_tensor_tensor(V2[st][:, ecs], t2[:],
                                                   float(1.0 / 1024.0),
                                                   V1[st][:, ecs], OP.mult, OP.add)
                act_phase(global_lns)
            vdefer = []
            _vexps = []
            for st in range(ST):
                sts = slice(st * 128, (st + 1) * 128)
                for ec in range(2):
                    ecs = slice(ec * 384, (ec + 1) * 384)
                    psm = s1p.tile([128, 384], f32, tag="vmu", name="vmu", bufs=2)
                    for ft in range(ET):
                        nc.tensor.matmul(psm[:], xmT[ft][:, sts],
                                         wt[ft][:, ecs],
                                         start=(ft == 0), stop=(ft == ET - 1))
                    h0 = ec * 6
                    dst = muv1[st][:, h0 * 65:(h0 + 6) * 65] \
                        .rearrange("p (a b) -> p a b", b=65)[:, :, 0:64]
                    nc.vector.tensor_copy(dst, psm[:].rearrange("p (a b) -> p a b", b=64))
                    t2 = s1v.tile([128, 384], bf16, tag=f"vsq{st % 4}_{ec}",
                                  name=f"vsq{st}_{ec}", bufs=1)
                    mv = muv1[st][:, h0 * 65:(h0 + 6) * 65]                         .rearrange("p (a b) -> p a b", b=65)[:, :, 0:64]
                    nc.vector.tensor_mul(t2[:].rearrange("p (a b) -> p a b", b=64),
                                         mv, mv)

                    pss = s1p.tile([128, 384], f32, tag="vsg", name="vsg", bufs=2)
                    for ft in range(ET):
                        nc.tensor.matmul(pss[:], xsT[ft][:, sts], wtb[ft][:, ecs],
                                         start=(ft == 0), stop=False)
                    nc.tensor.matmul(pss[:], combo[0:1, sts],
                                     wsig["v"][0:1, ecs],
                                     start=False, stop=True)
                    spv = s1v.tile([128, 384], bf16, tag=f"spv{st % 4}_{ec}",
                                   name=f"spv{st}_{ec}", bufs=1)
                    _vexps.append(nc.scalar.activation(spv[:], pss[:], AF.Exp))
                    vdefer.append((spv, st, ecs, t2))
                one_col = muv1[st][:].rearrange("p (a b) -> p a b", b=65)[:, :, 64:65]
                nc.vector.memset(one_col, 1.0)
                if st % 4 == 3:
                    flush_vdefer(vdefer)
                    vdefer = []
            s1v_cm.__exit__(None, None, None)

            # ---- head-scaled score operands (stacked per head into qb) ----
            s1hs_cm = tc.tile_pool(name="s1hs", bufs=1)
            s1hs = s1hs_cm.__enter__()
            for h in range(H):
                mq2 = s1hs.tile([64, LOC], bf16, tag=f"mq2_{h % 4}",
                                name=f"mq2_{h}", bufs=1)
                nc.scalar.activation(mq2[:], hs(mu_qT, h), AF.Square, scale=0.125)
                sq64 = s1hs.tile([64, LOC], bf16, tag=f"sq64_{h % 4}",
                                 name=f"sq64_{h}", bufs=1)
                nc.vector.tensor_scalar_mul(sq64[:], skq[h][64:128, 0:LOC],
                                            float(1.0 / 64.0))
                nc.vector.tensor_add(qb[h][0:64, :], mq2[:], sq64[:])
                nc.scalar.activation(qb[h][64:128, :], hs(mu_kT, h, slice(0, LOC)),
                                     AF.Square, scale=0.125)
            s1hs_cm.__exit__(None, None, None)

        # ================= stage 2: attention =================
        perB = ex.enter_context(tc.tile_pool(name="perB", bufs=1))
        mu_catT = [perB.tile([128, LOC], f32r, tag=f"mucat{i}", name=f"mucat{i}") for i in range(ET)]
        sg_craw = [perB.tile([128, LOC], bf16, tag=f"sgcraw{i}", name=f"sgcraw{i}") for i in range(ET)]
        sg_catT = [perB.tile([128, LOC], bf16, tag=f"sgcat{i}", name=f"sgcat{i}") for i in range(ET)]

        with tc.tile_pool(name="a_mur", bufs=2, space=PSUM) as p_mur, \
             tc.tile_pool(name="a_ms", bufs=2, space=PSUM) as p_ms, \
             tc.tile_pool(name="a_ss", bufs=2, space=PSUM) as p_ss, \
             tc.tile_pool(name="a_s12", bufs=2, space=PSUM) as p_s12, \
             tc.tile_pool(name="a_sb", bufs=2) as asb, \
             tc.tile_pool(name="a_e", bufs=2 * ST + 2) as aeb, \
             tc.tile_pool(name="a_sm", bufs=2) as asm, \
             tc.tile_pool(name="a_rv", bufs=1) as arv:

            def pass1_pair(h0, h1):
                Es = {h: [aeb.tile([128, 512], bf16, tag="E", name=f"E{h}_{kt}")
                          for kt in range(ST)] for h in (h0, h1)}
                murs = [p_mur.tile([65, 512], f32, tag="mur", name=f"mur{h}")
                        for h in (h0, h1)]
                for kt in range(ST):
                    kcs = slice(kt * 128, (kt + 1) * 128)
                    for i, h in enumerate((h0, h1)):
                        ms = p_ms.tile([128, 512], f32, tag="ms", name=f"ms{h}_{kt}")
                        nc.tensor.matmul(ms[:], hs(mu_kT, h, kcs), hs(mu_qT, h))
                        _s2acts.append(
                            nc.scalar.activation(Es[h][kt][:], ms[:], AF.Exp,
                                                 scale=0.125))
                        nc.tensor.matmul(murs[i][:],
                                         muv1[kt][:, h * 65:(h + 1) * 65],
                                         Es[h][kt][:], start=(kt == 0),
                                         stop=(kt == ST - 1))
                return Es, murs

            def norm(h, mur):
                r_sb = arv.tile([1, 512], f32, tag="r", name=f"r{h}")
                nc.scalar.copy(r_sb[:], mur[64:65, :])
                rscr = arv.tile([1, 512], f32, tag="rs", name=f"rs{h}")
                rinv = arv.tile([1, 512], f32, tag="ri", name=f"ri{h}")
                nc.vector.reciprocal_approx_accurate(rinv[:], r_sb[:], rscr[:])
                rinv2 = arv.tile([1, 512], f32, tag="ri2", name=f"ri2{h}")
                nc.vector.tensor_mul(rinv2[:], rinv[:], rinv[:])

                r_bf = arv.tile([1, 512], bf16, tag="rbf", name=f"rbf{h}")
                nc.vector.tensor_copy(r_bf[:], mur[64:65, :])
                R_b = asb.tile([128, 512], bf16, tag="Rb", name=f"Rb{h}")
                nc.gpsimd.partition_broadcast(R_b[:], r_bf[:])
                rinv_b = asb.tile([64, 512], f32, tag="rivb", name=f"rivb{h}")
                nc.gpsimd.partition_broadcast(rinv_b[:], rinv[:])
                rinv2_b = asb.tile([64, 512], f32, tag="riv2b", name=f"riv2b{h}")
                nc.gpsimd.partition_broadcast(rinv2_b[:], rinv2[:])
                return R_b, rinv_b, rinv2_b

            def pass2_pair(h0, h1, Es, murs, norms):
                psA = p_s12.tile([128, 512], f32, tag="s12", name=f"psA{h0}")
                psB = p_s12.tile([128, 512], f32, tag="s12", name=f"psB{h0}")
                for kt in range(ST):
                    for i, h in enumerate((h0, h1)):
                        rows = slice(i * 64, (i + 1) * 64)
                        pass2_kt(h, kt, Es[h], psA[rows, :], psB[rows, :],
                                 norms[i][0])
                for i, h in enumerate((h0, h1)):
                    rows = slice(i * 64, (i + 1) * 64)
                    pass2_post(h, psA[rows, :], psB[rows, :], murs[i],
                               norms[i][1], norms[i][2])

            def pass2_kt(h, kt, Et, pA, pB, R_b):
                    kcs = slice(kt * 128, (kt + 1) * 128)
                    ss = p_ss.tile([128, 512], f32, tag="ss", name=f"ss{h}_{kt}")
                    nc.tensor.matmul(ss[:], skq[h][:, kcs], qb[h][:])
                    Ht = asm.tile([128, 512], bf16, tag="Ht", name=f"Ht{h}_{kt}")
                    nc.vector.tensor_sub(Ht[:], R_b[:], Et[kt][:])
                    Mt = asm.tile([128, 512], bf16, tag="Mt", name=f"Mt{h}_{kt}")
                    nc.vector.tensor_mul(Mt[:], Et[kt][:], Ht[:])
                    M2t = asm.tile([128, 512], bf16, tag="M2t", name=f"M2t{h}_{kt}")
                    nc.scalar.activation(M2t[:], Mt[:], AF.Square)
                    E2t = asm.tile([128, 512], bf16, tag="E2t", name=f"E2t{h}_{kt}")
                    nc.scalar.activation(E2t[:], Et[kt][:], AF.Square)
                    Gt = asm.tile([128, 512], bf16, tag="Gt", name=f"Gt{h}_{kt}")
                    nc.vector.scalar_tensor_tensor(Gt[:], ss[:], INV_DWDIM, M2t[:],
                                                   OP.mult, OP.mult)
                    nc.tensor.matmul(pA, V1[kt][:, h * 64:(h + 1) * 64],
                                     E2t[:], start=(kt == 0), stop=(kt == ST - 1))
                    nc.tensor.matmul(pB, V2[kt][:, h * 64:(h + 1) * 64],
                                     Gt[:], start=(kt == 0), stop=(kt == ST - 1))

            def pass2_post(h, pA, pB, mur, rinv_b, rinv2_b):
                ta = asm.tile([64, 512], f32, tag="ta", name=f"ta{h}")
                nc.vector.tensor_mul(ta[:], pB, rinv2_b[:])
                tb = asm.tile([64, 512], f32, tag="tb", name=f"tb{h}")
                nc.vector.tensor_add(tb[:], ta[:], pA)
                nc.vector.tensor_mul(hs(sg_craw, h), tb[:], rinv2_b[:])
                nc.vector.tensor_mul(hs(mu_catT, h), mur[0:64, :], rinv_b[:])

            _s2acts = []
            spc_l = []
            _spcexp = []
            for hp in range(H // 2):
                h0, h1 = 2 * hp, 2 * hp + 1
                E0, m0 = pass1_pair(h0, h1)
                n0 = norm(h0, m0[0])
                n1 = norm(h1, m0[1])
                pass2_pair(h0, h1, E0, m0, (n0, n1))
                spc = perB.tile([128, 512], bf16, tag=f"spc{hp}", name=f"spc{hp}",
                                bufs=1)
                _spcexp.append(nc.scalar.activation(spc[:], sg_craw[hp][:], AF.Exp))
                spc_l.append(spc)
            act_phase(_s2acts[:1])
            act_phase(_spcexp)

        # ================= stage 3: output projection =================
        with tc.tile_pool(name="s3w", bufs=1) as s3w, \
             tc.tile_pool(name="s3p", bufs=3, space=PSUM) as s3p, \
             tc.tile_pool(name="s3s", bufs=2) as s3s, \
             tc.tile_pool(name="s3o", bufs=2) as s3o:

            wo_big = [s3w.tile([128, 3 * E], f32r, tag=f"woB{j}", name=f"woB{j}")
                      for j in range(2)]
            wos_big = s3w.tile([128, ET * E], bf16, tag="wosB", name="wosB")
            wo = [wo_big[i // 3][:, (i % 3) * E:(i % 3 + 1) * E] for i in range(ET)]
            wos = [wos_big[:, i * E:(i + 1) * E] for i in range(ET)]
            wr = w_d["o"].rearrange("(t p) e -> p t e", p=128)
            wbr = wsq_d["o"].rearrange("(t p) e -> p t e", p=128)
            for j in range(2):
                nc.sync.dma_start(
                    wo_big[j][:].rearrange("p (t e) -> p t e", e=E),
                    wr[:, 3 * j:3 * j + 3, :])
            nc.sync.dma_start(
                wos_big[:].rearrange("p (t e) -> p t e", e=E), wbr[:])

            act_phase([nc.scalar.activation(sg_catT[i][:], spc_l[i][:], AF.Ln,
                                            bias=1.0) for i in range(ET)])

            csum = s3p.tile([1, 512], f32, tag="cso", name="cso", bufs=1)
            for i in range(ET):
                x2 = s3s.tile([128, 512], bf16, tag="x2o", name="x2o")
                nc.scalar.activation(x2[:], mu_catT[i][:], AF.Square,
                                     scale=float(1.0 / 768.0))
                nc.tensor.matmul(csum[:], ones_cb[:], x2[:], start=(i == 0), stop=False)
            for i in range(ET):
                nc.tensor.matmul(csum[:], o768_cb[:], sg_catT[i][:],
                                 start=False, stop=(i == ET - 1))
            nc.vector.tensor_copy(combo_o_t[:], csum[:])

            odefer = []
            sgmap = {}
            for qt in range(4):
                qcs = slice(qt * 128, (qt + 1) * 128)
                mu_sb = s3o.tile([128, E], f32, tag="mu_sb", name=f"mu_sb{qt}", bufs=2)
                sg_sb = None
                for ec in range(2):
                    ecs = slice(ec * 384, (ec + 1) * 384)
                    psm = s3p.tile([128, 384], f32, tag="omu", name="omu")
                    for ft in range(ET):
                        nc.tensor.matmul(psm[:], mu_catT[ft][:, qcs],
                                         wo[ft][:, ecs],
                                         start=(ft == 0), stop=(ft == ET - 1))
                    nc.vector.tensor_copy(mu_sb[:, ecs], psm[:])
                    pss = s3p.tile([128, 384], f32, tag="osg", name="osg")
                    for ft in range(ET):
                        nc.tensor.matmul(pss[:], sg_catT[ft][:, qcs], wos[ft][:, ecs],
                                         start=(ft == 0), stop=False)
                    nc.tensor.matmul(pss[:], combo_o[0:1, qcs],
                                     wsig["o"][0:1, ecs],
                                     start=False, stop=True)
                    sgr = s3s.tile([128, 384], bf16, tag=f"sgr{qt}_{ec}",
                                   name=f"sgr{qt}_{ec}", bufs=1)
                    nc.vector.tensor_copy(sgr[:], pss[:])
                    odefer.append((qt, ec, ecs, sgr, mu_sb, sg_sb))
                nc.sync.dma_start(muo_d[qcs, :], mu_sb[:])
            # batched double softplus over all 8 output tiles
            pa = {}
            _ph = []
            for qt, ec, ecs, sgr, mu_sb, sg_sb in odefer:
                a = s3s.tile([128, 384], bf16, tag=f"pa{qt}_{ec}",
                             name=f"pa{qt}_{ec}", bufs=1)
                _ph.append(nc.scalar.activation(a[:], sgr[:], AF.Exp))
                pa[(qt, ec)] = a
            act_phase(_ph)
            pb = {}
            _ph = []
            for qt, ec, ecs, sgr, mu_sb, sg_sb in odefer:
                b = s3s.tile([128, 384], bf16, tag=f"sgr{qt}_{ec}",
                             name=f"pb{qt}_{ec}", bufs=1)
                _ph.append(nc.scalar.activation(b[:], pa[(qt, ec)][:], AF.Ln, bias=1.0))
                pb[(qt, ec)] = b
            act_phase(_ph)
            pc = {}
            _ph = []
            for qt, ec, ecs, sgr, mu_sb, sg_sb in odefer:
                c = s3s.tile([128, 384], bf16, tag=f"pa{qt}_{ec}",
                             name=f"pc{qt}_{ec}", bufs=1)
                _ph.append(nc.scalar.activation(c[:], pb[(qt, ec)][:], AF.Exp))
                pc[(qt, ec)] = c
            act_phase(_ph)
            _ph = []
            for qt, ec, ecs, sgr, mu_sb, sg_sb in odefer:
                sgh = s3o.tile([128, 384], f32, tag="sgh", name=f"sgh{qt}_{ec}", bufs=2)
                _ph.append(nc.scalar.activation(sgh[:], pc[(qt, ec)][:], AF.Ln, bias=1.0))
                nc.sync.dma_start(sgo_d[qt * 128:(qt + 1) * 128, ecs], sgh[:])
            act_phase(_ph)

    nc.compile()
    return nc


def _get_program():
    if "nc" not in _CACHE:
        _CACHE["nc"] = _build_program()
    return _CACHE["nc"]


def kernel(mu_inputs, sigma_inputs, wq_mu, wq_sigma, wk_mu, wk_sigma,
           wv_mu, wv_sigma, wo_mu, wo_sigma):
    from concourse.bass_utils import run_bass_kernel_spmd

    nc = _get_program()
    bf = ml_dtypes.bfloat16
    w = {"q": wq_mu, "k": wk_mu, "v": wv_mu, "o": wo_mu}
    wsigma = {"q": wq_sigma, "k": wk_sigma, "v": wv_sigma, "o": wo_sigma}
    shared = {}
    for k in "qkvo":
        wm = np.ascontiguousarray(np.asarray(w[k], np.float32))
        shared[f"w{k}"] = wm
        shared[f"w{k}s"] = np.ascontiguousarray(
            ((wm.astype(np.float64) / np.sqrt(768.0)) ** 2).astype(np.float32)).astype(bf)
        shared[f"sig{k}"] = _softplus64(wsigma[k]).astype(np.float32) \
            .astype(bf).reshape(1, E)

    in_maps = []
    for core in range(NC):
        b, half = core // 2, core % 2
        xm = np.asarray(mu_inputs[b], np.float32)
        xs = np.asarray(sigma_inputs[b], np.float32)
        if half == 1:
            idx = np.r_[512:1024, 0:512]
            xm, xs = xm[idx], xs[idx]
        m = dict(shared)
        m["xmT"] = np.ascontiguousarray(xm.T)
        m["xsT"] = np.ascontiguousarray(xs.T).astype(bf)
        in_maps.append(m)

    res = run_bass_kernel_spmd(nc, in_maps, core_ids=list(range(NC)))
    _CACHE["last_res"] = res

    mu_out = np.zeros((B, S, E), np.float32)
    sig_out = np.zeros((B, S, E), np.float32)
    for core in range(NC):
        b, half = core // 2, core % 2
        rows = slice(half * 512, (half + 1) * 512)
        mu_out[b, rows] = res.results[core]["mu_out"]
        sig_out[b, rows] = res.results[core]["sig_out"]

    PRIOR = 0.01
    kl = 0.0
    for k in "qkvo":
        mu = np.asarray(w[k], np.float64)
        lv = np.asarray(wsigma[k], np.float64)
        t = (np.log(PRIOR) - 1.0 - lv + _softplus64(lv) / PRIOR + mu ** 2 / PRIOR)
        kl += 0.5 * t.mean()
    return mu_out, sig_out, np.float32(kl)


# revision 37
# speedup vs baseline: 1.0110x; 1.0110x over previous
"""Bayesian (VDP) multi-head self-attention forward on 8 Trainium2 NeuronCores.

Sharding: core = (batch b, query-half). The host permutes each batch's
sequence so the core-local 512 query rows come first (attention sums over
keys, so a consistent key permutation is a no-op). Each core computes its
512 output rows completely: Q/K/V variational linears, 12-head VDP
attention, and the output projection including softplus. Host glue is
concatenation plus the scalar KL term.

Device layout notes:
- Inputs arrive pre-transposed [768, S] so every matmul contracts over the
  partition dim with zero on-device transposes.
- Attention scores are produced directly in [k, q] orientation; softmax
  normalization is algebraically deferred to per-q post-scales applied to
  the [d, q]-oriented PV outputs (the 1/r powers commute with the k-sum).
- Sigma_weights = (W - W^2)^2 * sigma_score/(d*wdim) is computed as
  (E*(r-E))^2 * ss/(d*wdim) * r^-4 with E = exp(score/sqrt(d)) unnormalized.
- mu-path matmuls run in float32r (full-rate fp32), sigma path in bf16.
"""

import contextlib
import numpy as np
import ml_dtypes

B, S, E, H, d = 4, 1024, 768, 12, 64
LOC = 512
NC = 8
INV_DWDIM = float(1.0 / (64.0 * 1024.0))
ET = E // 128   # 6 embedding partition tiles
ST = S // 128   # 8 sequence partition tiles
HT = 2          # heads per 128-partition tile

_CACHE: dict = {}


def _softplus64(x):
    x = np.asarray(x, np.float64)
    return np.log1p(np.exp(-np.abs(x))) + np.maximum(x, 0.0)


def _build_program():
    import concourse.bass as bass
    import concourse.bacc as bacc
    import concourse.mybir as mybir
    import concourse.tile as tile

    f32 = mybir.dt.float32
    f32r = mybir.dt.float32r
    bf16 = mybir.dt.bfloat16
    AF = mybir.ActivationFunctionType
    OP = mybir.AluOpType
    PSUM = bass.MemorySpace.PSUM

    nc = bacc.Bacc("TRN2", target_bir_lowering=False, debug=False,
                   num_devices=NC)

    xmT_d = nc.dram_tensor("xmT", [E, S], f32r, kind="ExternalInput")
    xsT_d = nc.dram_tensor("xsT", [E, S], bf16, kind="ExternalInput")
    w_d = {k: nc.dram_tensor(f"w{k}", [E, E], f32r, kind="ExternalInput") for k in "qkvo"}
    wsq_d = {k: nc.dram_tensor(f"w{k}s", [E, E], bf16, kind="ExternalInput") for k in "qkvo"}
    sig_d = {k: nc.dram_tensor(f"sig{k}", [1, E], bf16, kind="ExternalInput")
             for k in "qkvo"}
    muo_d = nc.dram_tensor("mu_out", [LOC, E], f32, kind="ExternalOutput")
    sgo_d = nc.dram_tensor("sig_out", [LOC, E], f32, kind="ExternalOutput")

    _actchain = {"last": None}

    def act_phase(insts):
        """Order a batch of same-table-set ACT ops after the previous batch
        (scheduler-only edges) so walrus emits one table load per batch."""
        if not insts:
            return
        if _actchain["last"] is not None:
            prev = _actchain["last"].ins
            for bi in insts:
                tile.add_dep_helper(bi.ins, prev, sync=False, reason="act-set phase")
        _actchain["last"] = insts[-1]

    def hs(tiles, h, cols=None):
        """AP for head h (64 rows) of a [768, *]-as-6-tiles tensor."""
        t, p0 = h // HT, (h % HT) * 64
        ap = tiles[t]
        return ap[p0:p0 + 64, :] if cols is None else ap[p0:p0 + 64, cols]

    with nc.allow_low_precision(reason="f32r/bf16 compute by design"), \
         tile.TileContext(nc) as tc, contextlib.ExitStack() as ex:
        cst = ex.enter_context(tc.tile_pool(name="cst", bufs=1))

        ones_f = cst.tile([1, 128], f32, tag="ones_f", name="ones_f")
        nc.vector.memset(ones_f[:], 1.0)
        ones_cb = cst.tile([128, 1], bf16, tag="ones_cb", name="ones_cb")
        nc.vector.memset(ones_cb[:], 1.0)
        o768_cb = cst.tile([128, 1], bf16, tag="o768_cb", name="o768_cb")
        nc.vector.memset(o768_cb[:], float(1.0 / 768.0))
        wsig = {}
        for k in "qkvo":
            wsig[k] = cst.tile([1, E], bf16, tag=f"wsig{k}", name=f"wsig{k}")
            nc.sync.dma_start(wsig[k][:], sig_d[k][:])
        combo_t = cst.tile([1, S], bf16, tag="combo_t", name="combo_t")
        combo = combo_t[0:1, :]
        combo_o_t = cst.tile([1, LOC], bf16, tag="combo_o_t", name="combo_o_t")
        combo_o = combo_o_t[0:1, :]

        perA = ex.enter_context(tc.tile_pool(name="perA", bufs=1))
        mu_qT = [perA.tile([128, LOC], f32r, tag=f"muq{i}", name=f"muq{i}") for i in range(ET)]
        mu_kT = [perA.tile([128, S], f32r, tag=f"muk{i}", name=f"muk{i}") for i in range(ET)]
        # per-head stacked sigma operands: rows 0:64 = sg_k(h), 64:128 = sg_q(h)
        skq = [perA.tile([128, S], bf16, tag=f"skq{h}", name=f"skq{h}") for h in range(H)]
        # per-head stacked rhs: rows 0:64 = mu_q^2/64 + sg_q/64, 64:128 = mu_k^2/64
        qb = [perA.tile([128, LOC], bf16, tag=f"qb{h}", name=f"qb{h}") for h in range(H)]



        muv1 = [perA.tile([128, H * 65], bf16, tag=f"muv1_{i}", name=f"muv1_{i}") for i in range(ST)]
        V1 = [perA.tile([128, E], bf16, tag=f"V1_{i}", name=f"V1_{i}") for i in range(ST)]
        V2 = [perA.tile([128, E], bf16, tag=f"V2_{i}", name=f"V2_{i}") for i in range(ST)]

        # ================= stage 1: linears =================
        with tc.tile_pool(name="s1x", bufs=1) as s1x, \
             tc.tile_pool(name="s1w", bufs=1) as s1w, \
             tc.tile_pool(name="s1wb", bufs=1) as s1wb, \
             tc.tile_pool(name="s1p", bufs=1, space=PSUM) as s1p, \
             tc.tile_pool(name="s1s", bufs=3) as s1s:

            xm_big = [s1x.tile([128, 2 * S], f32r, tag=f"xmB{j}", name=f"xmB{j}")
                      for j in range(3)]
            xs_big = [s1x.tile([128, 3 * S], bf16, tag=f"xsB{j}", name=f"xsB{j}")
                      for j in range(2)]
            xmT = [xm_big[i // 2][:, (i % 2) * S:(i % 2 + 1) * S] for i in range(ET)]
            xsT = [xs_big[i // 3][:, (i % 3) * S:(i % 3 + 1) * S] for i in range(ET)]
            xmT_r = xmT_d.rearrange("(t p) s -> p t s", p=128)
            xsT_r = xsT_d.rearrange("(t p) s -> p t s", p=128)
            for j in range(3):
                nc.sync.dma_start(
                    xm_big[j][:].rearrange("p (t s) -> p t s", s=S),
                    xmT_r[:, 2 * j:2 * j + 2, :])
            for j in range(2):
                nc.sync.dma_start(
                    xs_big[j][:].rearrange("p (t s) -> p t s", s=S),
                    xsT_r[:, 3 * j:3 * j + 3, :])

            # combo[s] = sum_f (xm[f,s]/768)^2 + sum_f xs[f,s]/768
            for sc in range(2):
                scs = slice(sc * 512, (sc + 1) * 512)
                csum = s1p.tile([1, 512], f32, tag="csum", name="csum", bufs=1)
                for i in range(ET):
                    x2 = s1s.tile([128, 512], bf16, tag="x2", name="x2")
                    nc.scalar.activation(x2[:], xmT[i][:, scs], AF.Square,
                                         scale=float(1.0 / 768.0))
                    nc.tensor.matmul(csum[:], ones_cb[:], x2[:],
                                     start=(i == 0), stop=False)
                for i in range(ET):
                    nc.tensor.matmul(csum[:], o768_cb[:], xsT[i][:, scs],
                                     start=False, stop=(i == ET - 1))
                nc.vector.tensor_copy(combo[0:1, scs], csum[:])

            wt_big = [s1w.tile([128, 3 * E], f32r, tag=f"wB{j}", name=f"wB{j}")
                      for j in range(2)]
            wtb_big = s1wb.tile([128, ET * E], bf16, tag="wsB", name="wsB")
            wt = [wt_big[i // 3][:, (i % 3) * E:(i % 3 + 1) * E] for i in range(ET)]
            wtb = [wtb_big[:, i * E:(i + 1) * E] for i in range(ET)]

            def load_weights(k):
                wr = w_d[k].rearrange("(t p) e -> p t e", p=128)
                wbr = wsq_d[k].rearrange("(t p) e -> p t e", p=128)
                for j in range(2):
                    nc.sync.dma_start(
                        wt_big[j][:].rearrange("p (t e) -> p t e", e=E),
                        wr[:, 3 * j:3 * j + 3, :])
                nc.sync.dma_start(
                    wtb_big[:].rearrange("p (t e) -> p t e", e=E), wbr[:])

            # ---- Q/K transposed-out linears ----
            s1qk_cm = tc.tile_pool(name="s1qk", bufs=1)
            s1qk = s1qk_cm.__enter__()
            for k, mu_dst, nq in (("q", mu_qT, LOC // 512), ("k", mu_kT, S // 512)):
                load_weights(k)
                for et in range(ET):
                    ecs = slice(et * 128, (et + 1) * 128)
                    for sc in range(nq):
                        scs = slice(sc * 512, (sc + 1) * 512)
                        ps = s1p.tile([128, 512], f32, tag="lin", name="lin", bufs=3)
                        for ft in range(ET):
                            nc.tensor.matmul(ps[:], wt[ft][:, ecs],
                                             xmT[ft][:, scs],
                                             start=(ft == 0), stop=(ft == ET - 1))
                        nc.vector.tensor_copy(mu_dst[et][:, scs], ps[:])
                dst_half = slice(64, 128) if k == "q" else slice(0, 64)
                spx_l = []
                _exps = []
                for et in range(ET):
                    ecs = slice(et * 128, (et + 1) * 128)
                    for sc in range(S // 512):
                        scs = slice(sc * 512, (sc + 1) * 512)
                        ps = s1p.tile([128, 512], f32, tag="lin", name="lin", bufs=3)
                        for ft in range(ET):
                            nc.tensor.matmul(ps[:], wtb[ft][:, ecs], xsT[ft][:, scs],
                                             start=(ft == 0), stop=False)
                        nc.tensor.matmul(ps[:], wsig[k][0:1, ecs],
                                         combo[0:1, scs],
                                         start=False, stop=True)
                        spx = s1qk.tile([128, 512], bf16, tag=f"spx{et}_{sc}",
                                        name=f"spx{et}_{sc}", bufs=1)
                        _exps.append(nc.scalar.activation(spx[:], ps[:], AF.Exp))
                        spx_l.append((spx, et, scs))
                act_phase(_exps)
                _lns = []
                for spx, et, scs in spx_l:
                    for par in range(2):
                        _lns.append(nc.scalar.activation(
                            skq[2 * et + par][dst_half, scs],
                            spx[par * 64:(par + 1) * 64, :], AF.Ln, bias=1.0))
                act_phase(_lns)

            s1qk_cm.__exit__(None, None, None)

            # ---- V natural-out linears ----
            load_weights("v")
            s1v_cm = tc.tile_pool(name="s1v", bufs=1)
            s1v = s1v_cm.__enter__()

            def flush_vdefer(vdefer):
                global_lns = []
                act_phase(_vexps)
                _vexps.clear()
                for spv, st, ecs, t2 in vdefer:
                    sgv = s1s.tile([128, 384], bf16, tag="sgv", name="sgv", bufs=3)
                    global_lns.append(
                        nc.scalar.activation(sgv[:], spv[:], AF.Ln, bias=1.0))
                    nc.vector.tensor_scalar_mul(V1[st][:, ecs], sgv[:],
                                                float(1.0 / 1024.0))
                    nc.vector.scalar_tensor_tensor(V2[st][:, ecs], t2[:],
                                                   float(1.0 / 1024.0),
                                                   V1[st][:, ecs], OP.mult, OP.add)
                act_phase(global_lns)
            vdefer = []
            _vexps = []
            for st in range(ST):
                sts = slice(st * 128, (st + 1) * 128)
                for ec in range(2):
                    ecs = slice(ec * 384, (ec + 1) * 384)
                    psm = s1p.tile([128, 384], f32, tag="vmu", name="vmu", bufs=2)
                    for ft in range(ET):
                        nc.tensor.matmul(psm[:], xmT[ft][:, sts],
                                         wt[ft][:, ecs],
                                         start=(ft == 0), stop=(ft == ET - 1))
                    h0 = ec * 6
                    dst = muv1[st][:, h0 * 65:(h0 + 6) * 65] \
                        .rearrange("p (a b) -> p a b", b=65)[:, :, 0:64]
                    nc.vector.tensor_copy(dst, psm[:].rearrange("p (a b) -> p a b", b=64))
                    t2 = s1v.tile([128, 384], bf16, tag=f"vsq{st % 4}_{ec}",
                                  name=f"vsq{st}_{ec}", bufs=1)
                    mv = muv1[st][:, h0 * 65:(h0 + 6) * 65]                         .rearrange("p (a b) -> p a b", b=65)[:, :, 0:64]
                    nc.vector.tensor_mul(t2[:].rearrange("p (a b) -> p a b", b=64),
                                         mv, mv)

                    pss = s1p.tile([128, 384], f32, tag="vsg", name="vsg", bufs=2)
                    for ft in range(ET):
                        nc.tensor.matmul(pss[:], xsT[ft][:, sts], wtb[ft][:, ecs],
                                         start=(ft == 0), stop=False)
                    nc.tensor.matmul(pss[:], combo[0:1, sts],
                                     wsig["v"][0:1, ecs],
                                     start=False, stop=True)
                    spv = s1v.tile([128, 384], bf16, tag=f"spv{st % 4}_{ec}",
                                   name=f"spv{st}_{ec}", bufs=1)
                    _vexps.append(nc.scalar.activation(spv[:], pss[:], AF.Exp))
                    vdefer.append((spv, st, ecs, t2))
                one_col = muv1[st][:].rearrange("p (a b) -> p a b", b=65)[:, :, 64:65]
                nc.vector.memset(one_col, 1.0)
                if st % 4 == 3:
                    flush_vdefer(vdefer)
                    vdefer = []
            s1v_cm.__exit__(None, None, None)

            # ---- head-scaled score operands (stacked per head into qb) ----
            s1hs_cm = tc.tile_pool(name="s1hs", bufs=1)
            s1hs = s1hs_cm.__enter__()
            for h in range(H):
                mq2 = s1hs.tile([64, LOC], bf16, tag=f"mq2_{h % 4}",
                                name=f"mq2_{h}", bufs=1)
                nc.scalar.activation(mq2[:], hs(mu_qT, h), AF.Square, scale=0.125)
                sq64 = s1hs.tile([64, LOC], bf16, tag=f"sq64_{h % 4}",
                                 name=f"sq64_{h}", bufs=1)
                nc.vector.tensor_scalar_mul(sq64[:], skq[h][64:128, 0:LOC],
                                            float(1.0 / 64.0))
                nc.vector.tensor_add(qb[h][0:64, :], mq2[:], sq64[:])
                nc.scalar.activation(qb[h][64:128, :], hs(mu_kT, h, slice(0, LOC)),
                                     AF.Square, scale=0.125)
            s1hs_cm.__exit__(None, None, None)

        # ================= stage 2: attention =================
        perB = ex.enter_context(tc.tile_pool(name="perB", bufs=1))
        mu_catT = [perB.tile([128, LOC], f32r, tag=f"mucat{i}", name=f"mucat{i}") for i in range(ET)]
        sg_craw = [perB.tile([128, LOC], bf16, tag=f"sgcraw{i}", name=f"sgcraw{i}") for i in range(ET)]
        sg_catT = [perB.tile([128, LOC], bf16, tag=f"sgcat{i}", name=f"sgcat{i}") for i in range(ET)]

        with tc.tile_pool(name="a_mur", bufs=2, space=PSUM) as p_mur, \
             tc.tile_pool(name="a_ms", bufs=2, space=PSUM) as p_ms, \
             tc.tile_pool(name="a_ss", bufs=2, space=PSUM) as p_ss, \
             tc.tile_pool(name="a_s12", bufs=2, space=PSUM) as p_s12, \
             tc.tile_pool(name="a_sb", bufs=2) as asb, \
             tc.tile_pool(name="a_e", bufs=2 * ST + 2) as aeb, \
             tc.tile_pool(name="a_sm", bufs=2) as asm, \
             tc.tile_pool(name="a_rv", bufs=1) as arv:

            def pass1_pair(h0, h1):
                Es = {h: [aeb.tile([128, 512], bf16, tag="E", name=f"E{h}_{kt}")
                          for kt in range(ST)] for h in (h0, h1)}
                murs = [p_mur.tile([65, 512], f32, tag="mur", name=f"mur{h}")
                        for h in (h0, h1)]
                for kt in range(ST):
                    kcs = slice(kt * 128, (kt + 1) * 128)
                    for i, h in enumerate((h0, h1)):
                        ms = p_ms.tile([128, 512], f32, tag="ms", name=f"ms{h}_{kt}")
                        nc.tensor.matmul(ms[:], hs(mu_kT, h, kcs), hs(mu_qT, h))
                        _s2acts.append(
                            nc.scalar.activation(Es[h][kt][:], ms[:], AF.Exp,
                                                 scale=0.125))
                        nc.tensor.matmul(murs[i][:],
                                         muv1[kt][:, h * 65:(h + 1) * 65],
                                         Es[h][kt][:], start=(kt == 0),
                                         stop=(kt == ST - 1))
                return Es, murs

            def norm(h, mur):
                r_sb = arv.tile([1, 512], f32, tag="r", name=f"r{h}")
                nc.scalar.copy(r_sb[:], mur[64:65, :])
                rscr = arv.tile([1, 512], f32, tag="rs", name=f"rs{h}")
                rinv = arv.tile([1, 512], f32, tag="ri", name=f"ri{h}")
                nc.vector.reciprocal_approx_accurate(rinv[:], r_sb[:], rscr[:])
                rinv2 = arv.tile([1, 512], f32, tag="ri2", name=f"ri2{h}")
                nc.vector.tensor_mul(rinv2[:], rinv[:], rinv[:])

                r_bf = arv.tile([1, 512], bf16, tag="rbf", name=f"rbf{h}")
                nc.vector.tensor_copy(r_bf[:], mur[64:65, :])
                R_b = asb.tile([128, 512], bf16, tag="Rb", name=f"Rb{h}")
                nc.gpsimd.partition_broadcast(R_b[:], r_bf[:])
                rinv_b = asb.tile([64, 512], f32, tag="rivb", name=f"rivb{h}")
                nc.gpsimd.partition_broadcast(rinv_b[:], rinv[:])
                rinv2_b = asb.tile([64, 512], f32, tag="riv2b", name=f"riv2b{h}")
                nc.gpsimd.partition_broadcast(rinv2_b[:], rinv2[:])
                return R_b, rinv_b, rinv2_b

            def pass2_pair(h0, h1, Es, murs, norms):
                psA = p_s12.tile([128, 512], f32, tag="s12", name=f"psA{h0}")
                psB = p_s12.tile([128, 512], f32, tag="s12", name=f"psB{h0}")
                for kt in range(ST):
                    for i, h in enumerate((h0, h1)):
                        rows = slice(i * 64, (i + 1) * 64)
                        pass2_kt(h, kt, Es[h], psA[rows, :], psB[rows, :],
                                 norms[i][0])
                for i, h in enumerate((h0, h1)):
                    rows = slice(i * 64, (i + 1) * 64)
                    pass2_post(h, psA[rows, :], psB[rows, :], murs[i],
                               norms[i][1], norms[i][2])

            def pass2_kt(h, kt, Et, pA, pB, R_b):
                    kcs = slice(kt * 128, (kt + 1) * 128)
                    ss = p_ss.tile([128, 512], f32, tag="ss", name=f"ss{h}_{kt}")
                    nc.tensor.matmul(ss[:], skq[h][:, kcs], qb[h][:])
                    Ht = asm.tile([128, 512], bf16, tag="Ht", name=f"Ht{h}_{kt}")
                    nc.vector.tensor_sub(Ht[:], R_b[:], Et[kt][:])
                    Mt = asm.tile([128, 512], bf16, tag="Mt", name=f"Mt{h}_{kt}")
                    nc.vector.tensor_mul(Mt[:], Et[kt][:], Ht[:])
                    M2t = asm.tile([128, 512], bf16, tag="M2t", name=f"M2t{h}_{kt}")
                    nc.scalar.activation(M2t[:], Mt[:], AF.Square)
                    E2t = asm.tile([128, 512], bf16, tag="E2t", name=f"E2t{h}_{kt}")
                    nc.scalar.activation(E2t[:], Et[kt][:], AF.Square)
                    Gt = asm.tile([128, 512], bf16, tag="Gt", name=f"Gt{h}_{kt}")
                    nc.vector.scalar_tensor_tensor(Gt[:], ss[:], INV_DWDIM, M2t[:],
                                                   OP.mult, OP.mult)
                    nc.tensor.matmul(pA, V1[kt][:, h * 64:(h + 1) * 64],
                                     E2t[:], start=(kt == 0), stop=(kt == ST - 1))
                    nc.tensor.matmul(pB, V2[kt][:, h * 64:(h + 1) * 64],
                                     Gt[:], start=(kt == 0), stop=(kt == ST - 1))

            def pass2_post(h, pA, pB, mur, rinv_b, rinv2_b):
                ta = asm.tile([64, 512], f32, tag="ta", name=f"ta{h}")
                nc.vector.tensor_mul(ta[:], pB, rinv2_b[:])
                tb = asm.tile([64, 512], f32, tag="tb", name=f"tb{h}")
                nc.vector.tensor_add(tb[:], ta[:], pA)
                nc.vector.tensor_mul(hs(sg_craw, h), tb[:], rinv2_b[:])
                nc.vector.tensor_mul(hs(mu_catT, h), mur[0:64, :], rinv_b[:])

            _s2acts = []
            for hp in range(H // 2):
                h0, h1 = 2 * hp, 2 * hp + 1
                E0, m0 = pass1_pair(h0, h1)
                n0 = norm(h0, m0[0])
                n1 = norm(h1, m0[1])
                pass2_pair(h0, h1, E0, m0, (n0, n1))
            act_phase(_s2acts[:1])

        # ================= stage 3: output projection =================
        with tc.tile_pool(name="s3w", bufs=1) as s3w, \
             tc.tile_pool(name="s3p", bufs=3, space=PSUM) as s3p, \
             tc.tile_pool(name="s3s", bufs=2) as s3s, \
             tc.tile_pool(name="s3o", bufs=2) as s3o:

            wo_big = [s3w.tile([128, 3 * E], f32r, tag=f"woB{j}", name=f"woB{j}")
                      for j in range(2)]
            wos_big = s3w.tile([128, ET * E], bf16, tag="wosB", name="wosB")
            wo = [wo_big[i // 3][:, (i % 3) * E:(i % 3 + 1) * E] for i in range(ET)]
            wos = [wos_big[:, i * E:(i + 1) * E] for i in range(ET)]
            wr = w_d["o"].rearrange("(t p) e -> p t e", p=128)
            wbr = wsq_d["o"].rearrange("(t p) e -> p t e", p=128)
            for j in range(2):
                nc.sync.dma_start(
                    wo_big[j][:].rearrange("p (t e) -> p t e", e=E),
                    wr[:, 3 * j:3 * j + 3, :])
            nc.sync.dma_start(
                wos_big[:].rearrange("p (t e) -> p t e", e=E), wbr[:])

            spc_l = []
            _swexp = []
            for i in range(ET):
                spc = s3s.tile([128, 512], bf16, tag=f"spc{i}", name=f"spc{i}", bufs=1)
                _swexp.append(nc.scalar.activation(spc[:], sg_craw[i][:], AF.Exp))
                spc_l.append(spc)
            act_phase(_swexp)
            act_phase([nc.scalar.activation(sg_catT[i][:], spc_l[i][:], AF.Ln,
                                            bias=1.0) for i in range(ET)])

            csum = s3p.tile([1, 512], f32, tag="cso", name="cso", bufs=1)
            for i in range(ET):
                x2 = s3s.tile([128, 512], bf16, tag="x2o", name="x2o")
                nc.scalar.activation(x2[:], mu_catT[i][:], AF.Square,
                                     scale=float(1.0 / 768.0))
                nc.tensor.matmul(csum[:], ones_cb[:], x2[:], start=(i == 0), stop=False)
            for i in range(ET):
                nc.tensor.matmul(csum[:], o768_cb[:], sg_catT[i][:],
                                 start=False, stop=(i == ET - 1))
            nc.vector.tensor_copy(combo_o_t[:], csum[:])

            odefer = []
            sgmap = {}
            for qt in range(4):
                qcs = slice(qt * 128, (qt + 1) * 128)
                mu_sb = s3o.tile([128, E], f32, tag="mu_sb", name=f"mu_sb{qt}", bufs=2)
                sg_sb = None
                for ec in range(2):
                    ecs = slice(ec * 384, (ec + 1) * 384)
                    psm = s3p.tile([128, 384], f32, tag="omu", name="omu")
                    for ft in range(ET):
                        nc.tensor.matmul(psm[:], mu_catT[ft][:, qcs],
                                         wo[ft][:, ecs],
                                         start=(ft == 0), stop=(ft == ET - 1))
                    nc.vector.tensor_copy(mu_sb[:, ecs], psm[:])
                    pss = s3p.tile([128, 384], f32, tag="osg", name="osg")
                    for ft in range(ET):
                        nc.tensor.matmul(pss[:], sg_catT[ft][:, qcs], wos[ft][:, ecs],
                                         start=(ft == 0), stop=False)
                    nc.tensor.matmul(pss[:], combo_o[0:1, qcs],
                                     wsig["o"][0:1, ecs],
                                     start=False, stop=True)
                    sgr = s3s.tile([128, 384], bf16, tag=f"sgr{qt}_{ec}",
                                   name=f"sgr{qt}_{ec}", bufs=1)
                    nc.vector.tensor_copy(sgr[:], pss[:])
                    odefer.append((qt, ec, ecs, sgr, mu_sb, sg_sb))
                nc.sync.dma_start(muo_d[qcs, :], mu_sb[:])
            # batched double softplus over all 8 output tiles
            pa = {}
            _ph = []
            for qt, ec, ecs, sgr, mu_sb, sg_sb in odefer:
                a = s3s.tile([128, 384], bf16, tag=f"pa{qt}_{ec}",
                             name=f"pa{qt}_{ec}", bufs=1)
                _ph.append(nc.scalar.activation(a[:], sgr[:], AF.Exp))
                pa[(qt, ec)] = a
            act_phase(_ph)
            pb = {}
            _ph = []
            for qt, ec, ecs, sgr, mu_sb, sg_sb in odefer:
                b = s3s.tile([128, 384], bf16, tag=f"sgr{qt}_{ec}",
                             name=f"pb{qt}_{ec}", bufs=1)
                _ph.append(nc.scalar.activation(b[:], pa[(qt, ec)][:], AF.Ln, bias=1.0))
                pb[(qt, ec)] = b
            act_phase(_ph)
            pc = {}
            _ph = []
            for qt, ec, ecs, sgr, mu_sb, sg_sb in odefer:
                c = s3s.tile([128, 384], bf16, tag=f"pa{qt}_{ec}",
                             name=f"pc{qt}_{ec}", bufs=1)
                _ph.append(nc.scalar.activation(c[:], pb[(qt, ec)][:], AF.Exp))
                pc[(qt, ec)] = c
            act_phase(_ph)
            _ph = []
            for qt, ec, ecs, sgr, mu_sb, sg_sb in odefer:
                sgh = s3o.tile([128, 384], f32, tag="sgh", name=f"sgh{qt}_{ec}", bufs=2)
                _ph.append(nc.scalar.activation(sgh[:], pc[(qt, ec)][:], AF.Ln, bias=1.0))
                nc.sync.dma_start(sgo_d[qt * 128:(qt + 1) * 128, ecs], sgh[:])
            act_phase(_ph)

    nc.compile()
    return nc


def _get_program():
    if "nc" not in _CACHE:
        _CACHE["nc"] = _build_program()
    return _CACHE["nc"]


def kernel(mu_inputs, sigma_inputs, wq_mu, wq_sigma, wk_mu, wk_sigma,
           wv_mu, wv_sigma, wo_mu, wo_sigma):
    from concourse.bass_utils import run_bass_kernel_spmd

    nc = _get_program()
    bf = ml_dtypes.bfloat16
    w = {"q": wq_mu, "k": wk_mu, "v": wv_mu, "o": wo_mu}
    wsigma = {"q": wq_sigma, "k": wk_sigma, "v": wv_sigma, "o": wo_sigma}
    shared = {}
    for k in "qkvo":
        wm = np.ascontiguousarray(np.asarray(w[k], np.float32))
        shared[f"w{k}"] = wm
        shared[f"w{k}s"] = np.ascontiguousarray(
            ((wm.astype(np.float64) / np.sqrt(768.0)) ** 2).astype(np.float32)).astype(bf)
        shared[f"sig{k}"] = _softplus64(wsigma[k]).astype(np.float32) \
            .astype(bf).reshape(1, E)

    in_maps = []
    for core in range(NC):
        b, half = core // 2, core % 2
        xm = np.asarray(mu_inputs[b], np.float32)
        xs = np.asarray(sigma_inputs[b], np.float32)
        if half == 1:
            idx = np.r_[512:1024, 0:512]
            xm, xs = xm[idx], xs[idx]
        m = dict(shared)
        m["xmT"] = np.ascontiguousarray(xm.T)
        m["xsT"] = np.ascontiguousarray(xs.T).astype(bf)
        in_maps.append(m)

    res = run_bass_kernel_spmd(nc, in_maps, core_ids=list(range(NC)))
    _CACHE["last_res"] = res

    mu_out = np.zeros((B, S, E), np.float32)
    sig_out = np.zeros((B, S, E), np.float32)
    for core in range(NC):
        b, half = core // 2, core % 2
        rows = slice(half * 512, (half + 1) * 512)
        mu_out[b, rows] = res.results[core]["mu_out"]
        sig_out[b, rows] = res.results[core]["sig_out"]

    PRIOR = 0.01
    kl = 0.0
    for k in "qkvo":
        mu = np.asarray(w[k], np.float64)
        lv = np.asarray(wsigma[k], np.float64)
        t = (np.log(PRIOR) - 1.0 - lv + _softplus64(lv) / PRIOR + mu ** 2 / PRIOR)
        kl += 0.5 * t.mean()
    return mu_out, sig_out, np.float32(kl)
